# revision 10
# baseline (speedup 1.0000x reference)
"""Trainium2 Bass kernel for coverage-attention (Bahdanau-style with coverage).

Reference computation (per batch row b):
  proj_sum[s,h] = enc_h[b,s,:] @ W_enc[:,h] + (dec_h[b] @ W_dec)[h] + cov[b,s]*w_cov[h]
  e[s]     = sum_h tanh(proj_sum[s,h] + bias) * v[h]
  a        = softmax(where(mask, e, -1e9))
  covloss  = sum_s min(a, cov)
  h_star   = sum_s a[s] * enc_h[b,s,:]
  new_cov  = cov + a

Strategy: data-parallel over batch (64 -> 8 per NeuronCore). Compute in bf16
on the TensorEngine (PSUM accumulates f32). The host pre-shards and
pre-marshals layouts: enc_h is shipped both natural [b,s,e] (for h_star) and
transposed [b,e,s] (for the projection contraction over e), already cast to
bf16. The dec-projection + bias and the coverage outer-product are folded
into the matmul chain as augmented rank-2 terms. proj is computed transposed
([h_part, s_free]) so the v-weighted tanh reduction is a TensorE matmul.
"""

import sys

sys.path.insert(0, "/opt/trn_rl_repo")

import numpy as np
import ml_dtypes

import concourse.bass as bass
import concourse.mybir as mybir
import concourse.tile as tile
from concourse import bacc
from concourse.bass_utils import run_bass_kernel_spmd

B, S, E, H = 64, 1024, 1024, 512  # E = 2H
NCORES = 8
BL = B // NCORES  # 8 local batch rows per core
KD = 640  # padded contraction dim for dec proj (512 + 1 bias row, padded to 5*128)

BF16 = mybir.dt.bfloat16
F32 = mybir.dt.float32
AF = mybir.ActivationFunctionType
ALU = mybir.AluOpType
AX = mybir.AxisListType

_graph = None
LAST_RUN = None  # BassKernelResults of the most recent run (exec_time_ns when traced)


def _build_graph():
    nc = bacc.Bacc()
    P = nc.declare_dram_parameter
    encT = P("encT", [BL, E, S], BF16, False)
    encN = P("encN", [BL, S, E], BF16, False)
    wenc = P("wenc", [E, H], BF16, False)
    wdeca = P("wdeca", [KD, H], BF16, False)       # rows 0..511 W_dec, 512 bias, rest 0
    dechTa = P("dechTa", [KD, BL], BF16, False)    # rows 0..511 dec_h.T, 512 ones, rest 0
    vcol = P("vcol", [H], BF16, False)
    wcov8 = P("wcov8", [BL * H], BF16, False)      # w_cov tiled BL times
    ones8k = P("ones8k", [BL * S], BF16, False)
    covb = P("covb", [BL, S], BF16, False)
    covf = P("covf", [BL, S], F32, False)
    emul = P("emul", [BL, S], F32, False)          # mask as f32
    eadd = P("eadd", [BL, S], F32, False)          # (mask-1)*1e9
    o_a = P("o_a", [BL, S], F32, True)
    o_h = P("o_h", [BL, E], F32, True)
    o_c = P("o_c", [BL, S], F32, True)
    o_l = P("o_l", [BL, 1], F32, True)

    with tile.TileContext(nc) as tc:
        with (
            tc.tile_pool(name="wp", bufs=1) as wp,
            tc.tile_pool(name="bigp", bufs=2) as bigp,
            tc.tile_pool(name="workp", bufs=3) as workp,
            tc.tile_pool(name="rowp", bufs=2) as rowp,
            tc.tile_pool(name="psum", bufs=1, space="PSUM") as pp,
            tc.tile_pool(name="dramp", bufs=2, space="DRAM") as dp,
        ):
            # ---- persistent weights / small tensors ----
            wenc_sb = wp.tile([128, 8, H], BF16)
            nc.sync.dma_start(wenc_sb[:], wenc[:].rearrange("(c p) h -> p c h", p=128))
            wdec_sb = wp.tile([128, 5, H], BF16)
            nc.sync.dma_start(wdec_sb[:], wdeca[:].rearrange("(c p) h -> p c h", p=128))
            dech_sb = wp.tile([128, 5, BL], BF16)
            nc.sync.dma_start(dech_sb[:], dechTa[:].rearrange("(c p) b -> p c b", p=128))
            v_sb = wp.tile([128, 4], BF16)
            nc.sync.dma_start(v_sb[:], vcol[:].rearrange("(c p) -> p c", p=128))

            augL = wp.tile([2, BL * H], BF16)   # row0 dec proj (filled below), row1 w_cov
            augR = wp.tile([2, BL * S], BF16)   # row0 ones, row1 coverage
            nc.sync.dma_start(augL[1:2, :], wcov8[None, :])
            nc.sync.dma_start(augR[0:1, :], ones8k[None, :])
            nc.sync.dma_start(augR[1:2, :], covb[:].rearrange("b s -> (b s)")[None, :])

            # ---- dec projection (+bias): decp[b,h] = dec_h[b] @ W_dec + bias ----
            dps = pp.tile([BL, H], F32, tag="acc", bufs=3)
            for k in range(5):
                nc.tensor.matmul(
                    dps[:], dech_sb[:, k, :], wdec_sb[:, k, :],
                    start=(k == 0), stop=(k == 4),
                )
            decb = wp.tile([BL, H], BF16)
            nc.any.tensor_copy(decb[:], dps[:])
            for b in range(BL):
                nc.sync.dma_start(augL[0:1, b * H:(b + 1) * H], decb[b:b + 1, :])

            # ---- main per-batch pipeline ----
            for b in range(BL):
                # transposed encoder slab for this batch: [e_part, s_free]
                et = bigp.tile([128, 8, S], BF16, tag="encT")
                for e in range(8):
                    nc.sync.dma_start(et[:, e, :], encT[b, e * 128:(e + 1) * 128, :])

                e_row = rowp.tile([1, S], F32, tag="erow")
                for sh in range(2):
                    eps = pp.tile([1, 512], F32, tag="eps", bufs=2)
                    for h in range(4):
                        prj = pp.tile([128, 512], F32, tag="prj", bufs=3)
                        for e in range(8):
                            nc.tensor.matmul(
                                prj[:],
                                wenc_sb[:, e, h * 128:(h + 1) * 128],
                                et[:, e, sh * 512:(sh + 1) * 512],
                                start=(e == 0), stop=False,
                            )
                        nc.tensor.matmul(
                            prj[:],
                            augL[:, b * H + h * 128: b * H + (h + 1) * 128],
                            augR[:, b * S + sh * 512: b * S + (sh + 1) * 512],
                            start=False, stop=True,
                        )
                        th = workp.tile([128, 512], BF16, tag="tanh")
                        nc.scalar.activation(th[:], prj[:], AF.Tanh)
                        nc.tensor.matmul(
                            eps[:], v_sb[:, h:h + 1], th[:],
                            start=(h == 0), stop=(h == 3),
                        )
                    nc.vector.tensor_copy(e_row[:, sh * 512:(sh + 1) * 512], eps[:])

                # masked softmax over the [1, S] score row
                emul_r = rowp.tile([1, S], F32, tag="emul_r")
                nc.sync.dma_start(emul_r[:], emul[b:b + 1, :])
                eadd_r = rowp.tile([1, S], F32, tag="eadd_r")
                nc.sync.dma_start(eadd_r[:], eadd[b:b + 1, :])
                covf_r = rowp.tile([1, S], F32, tag="covf_r")
                nc.sync.dma_start(covf_r[:], covf[b:b + 1, :])
                em = rowp.tile([1, S], F32, tag="em")
                nc.vector.tensor_tensor(em[:], e_row[:], emul_r[:], op=ALU.mult)
                nc.vector.tensor_tensor(em[:], em[:], eadd_r[:], op=ALU.add)
                mx = rowp.tile([1, 1], F32, tag="mx")
                nc.vector.reduce_max(mx[:], em[:], axis=AX.X)
                nc.vector.tensor_scalar_mul(mx[:], mx[:], -1.0)
                pr = rowp.tile([1, S], F32, tag="pr")
                nc.scalar.activation(pr[:], em[:], AF.Exp, bias=mx[:, 0:1], scale=1.0)
                sm = rowp.tile([1, 1], F32, tag="sm")
                nc.vector.reduce_sum(sm[:], pr[:], axis=AX.X)
                rv = rowp.tile([1, 1], F32, tag="rv")
                nc.vector.reciprocal(rv[:], sm[:])
                ar = rowp.tile([1, S], F32, tag="ar")
                nc.vector.tensor_scalar_mul(ar[:], pr[:], rv[:, 0:1])
                nc.sync.dma_start(o_a[b:b + 1, :], ar[:])

                ncv = rowp.tile([1, S], F32, tag="ncv")
                nc.vector.tensor_tensor(ncv[:], ar[:], covf_r[:], op=ALU.add)
                nc.sync.dma_start(o_c[b:b + 1, :], ncv[:])
                mnr = rowp.tile([1, S], F32, tag="mnr")
                nc.vector.tensor_tensor(mnr[:], ar[:], covf_r[:], op=ALU.min)
                cl = rowp.tile([1, 1], F32, tag="cl")
                nc.vector.reduce_sum(cl[:], mnr[:], axis=AX.X)
                nc.sync.dma_start(o_l[b:b + 1, :], cl[:])

                # attention weights to stationary-column layout via DRAM bounce
                ab = rowp.tile([1, S], BF16, tag="ab")
                nc.vector.tensor_copy(ab[:], ar[:])
                scr = dp.tile([S], BF16, tag="scr")
                nc.sync.dma_start(scr[None, :], ab[:])
                aT = workp.tile([128, BL], BF16, tag="aT")
                nc.sync.dma_start(aT[:], scr[:].rearrange("(c p) -> p c", p=128))

                # h_star[b] = a @ enc_h[b]
                hp0 = pp.tile([1, 512], F32, tag="acc", bufs=3)
                hp1 = pp.tile([1, 512], F32, tag="acc", bufs=3)
                for sc in range(8):
                    en = workp.tile([128, E], BF16, tag="encN", bufs=4)
                    nc.sync.dma_start(en[:], encN[b, sc * 128:(sc + 1) * 128, :])
                    nc.tensor.matmul(hp0[:], aT[:, sc:sc + 1], en[:, 0:512],
                                     start=(sc == 0), stop=(sc == 7))
                    nc.tensor.matmul(hp1[:], aT[:, sc:sc + 1], en[:, 512:1024],
                                     start=(sc == 0), stop=(sc == 7))
                hs = rowp.tile([1, E], F32, tag="hs")
                nc.any.tensor_copy(hs[:, 0:512], hp0[:])
                nc.any.tensor_copy(hs[:, 512:1024], hp1[:])
                nc.sync.dma_start(o_h[b:b + 1, :], hs[:])

    return nc


def get_graph():
    global _graph
    if _graph is None:
        _graph = _build_graph()
        if not _graph.is_finalized():
            _graph.finalize()
    return _graph


def kernel(enc_h, dec_h, coverage_vec, mask, W_enc, W_dec, w_cov, bias, v):
    bf = ml_dtypes.bfloat16
    enc_h = np.asarray(enc_h)
    dec_h = np.asarray(dec_h, dtype=np.float32)
    coverage_vec = np.asarray(coverage_vec, dtype=np.float32)
    mask_f = np.asarray(mask).astype(np.float32)
    W_enc = np.asarray(W_enc, dtype=np.float32)
    W_dec = np.asarray(W_dec, dtype=np.float32)
    w_cov = np.asarray(w_cov, dtype=np.float32)
    bias = np.asarray(bias, dtype=np.float32)
    v = np.asarray(v, dtype=np.float32)

    enc_bf = enc_h.astype(bf)  # [B, S, E]

    wdeca = np.zeros((KD, H), np.float32)
    wdeca[:H] = W_dec
    wdeca[H] = bias[0]
    wdeca = wdeca.astype(bf)
    wenc_bf = W_enc.astype(bf)
    vcol = v.astype(bf)
    wcov8 = np.tile(w_cov, BL).astype(bf)
    ones8k = np.ones(BL * S, bf)

    in_maps = []
    for c in range(NCORES):
        sl = slice(c * BL, (c + 1) * BL)
        encN_c = np.ascontiguousarray(enc_bf[sl])
        encT_c = np.ascontiguousarray(encN_c.transpose(0, 2, 1))
        dechTa = np.zeros((KD, BL), np.float32)
        dechTa[:H] = dec_h[sl].T
        dechTa[H] = 1.0
        cov_c = np.ascontiguousarray(coverage_vec[sl])
        m_c = np.ascontiguousarray(mask_f[sl])
        in_maps.append({
            "encT": encT_c,
            "encN": encN_c,
            "wenc": wenc_bf,
            "wdeca": wdeca,
            "dechTa": dechTa.astype(bf),
            "vcol": vcol,
            "wcov8": wcov8,
            "ones8k": ones8k,
            "covb": cov_c.astype(bf),
            "covf": cov_c,
            "emul": m_c,
            "eadd": ((m_c - 1.0) * 1e9).astype(np.float32),
        })

    run = run_bass_kernel_spmd(get_graph(), in_maps, list(range(NCORES)))
    global LAST_RUN
    LAST_RUN = run
    res = run.results

    a = np.concatenate([np.asarray(r["o_a"], np.float32) for r in res], axis=0)
    h_star = np.concatenate([np.asarray(r["o_h"], np.float32) for r in res], axis=0)
    new_cov = np.concatenate([np.asarray(r["o_c"], np.float32) for r in res], axis=0)
    covloss = np.concatenate(
        [np.asarray(r["o_l"], np.float32).reshape(BL) for r in res], axis=0
    )
    return (a, h_star, new_cov, covloss)


# revision 12
# speedup vs baseline: 1.1407x; 1.1407x over previous
"""Trainium2 Bass kernel for coverage-attention (Bahdanau-style with coverage).

Reference computation (per batch row b):
  proj_sum[s,h] = enc_h[b,s,:] @ W_enc[:,h] + (dec_h[b] @ W_dec)[h] + cov[b,s]*w_cov[h]
  e[s]     = sum_h tanh(proj_sum[s,h] + bias) * v[h]
  a        = softmax(where(mask, e, -1e9))
  covloss  = sum_s min(a, cov)
  h_star   = sum_s a[s] * enc_h[b,s,:]
  new_cov  = cov + a

Strategy: data-parallel over batch (64 -> 8 per NeuronCore). Compute in bf16
on the TensorEngine (PSUM accumulates f32). The host pre-shards and
pre-marshals layouts: enc_h is shipped both natural [b,s,e] (for h_star) and
transposed [b,e,s] (for the projection contraction over e), already cast to
bf16. The dec-projection + bias and the coverage outer-product are folded
into the matmul chain as augmented rank-2 terms. proj is computed transposed
([h_part, s_free]) so the v-weighted tanh reduction is a TensorE matmul.
"""

import sys

sys.path.insert(0, "/opt/trn_rl_repo")

import numpy as np
import ml_dtypes

import concourse.bass as bass
import concourse.mybir as mybir
import concourse.tile as tile
from concourse import bacc
from concourse.bass_utils import run_bass_kernel_spmd

B, S, E, H = 64, 1024, 1024, 512  # E = 2H
NCORES = 8
BL = B // NCORES  # 8 local batch rows per core
KD = 640  # padded contraction dim for dec proj (512 + 1 bias row, padded to 5*128)

BF16 = mybir.dt.bfloat16
F32 = mybir.dt.float32
AF = mybir.ActivationFunctionType
ALU = mybir.AluOpType
AX = mybir.AxisListType

_graph = None
LAST_RUN = None  # BassKernelResults of the most recent run (exec_time_ns when traced)


def _build_graph():
    nc = bacc.Bacc()
    P = nc.declare_dram_parameter
    encT = P("encT", [BL, E, S], BF16, False)
    encN = P("encN", [BL, S, E], BF16, False)
    wenc = P("wenc", [E, H], BF16, False)
    wdeca = P("wdeca", [KD, H], BF16, False)       # rows 0..511 W_dec, 512 bias, rest 0
    dechTa = P("dechTa", [KD, BL], BF16, False)    # rows 0..511 dec_h.T, 512 ones, rest 0
    vcol = P("vcol", [H], BF16, False)
    wcov8 = P("wcov8", [BL * H], BF16, False)      # w_cov tiled BL times
    ones8k = P("ones8k", [BL * S], BF16, False)
    covb = P("covb", [BL, S], BF16, False)
    covf = P("covf", [BL, S], F32, False)
    emul = P("emul", [BL, S], F32, False)          # mask as f32
    eadd = P("eadd", [BL, S], F32, False)          # (mask-1)*1e9
    o_a = P("o_a", [BL, S], F32, True)
    o_h = P("o_h", [BL, E], F32, True)
    o_c = P("o_c", [BL, S], F32, True)
    o_l = P("o_l", [BL, 1], F32, True)

    with tile.TileContext(nc) as tc:
        with (
            tc.tile_pool(name="wp", bufs=1) as wp,
            tc.tile_pool(name="bigp", bufs=2) as bigp,
            tc.tile_pool(name="workp", bufs=3) as workp,
            tc.tile_pool(name="rowp", bufs=2) as rowp,
            tc.tile_pool(name="psum", bufs=1, space="PSUM") as pp,
            tc.tile_pool(name="dramp", bufs=2, space="DRAM") as dp,
        ):
            # ---- persistent weights / small tensors ----
            wenc_sb = wp.tile([128, 8, H], BF16)
            nc.sync.dma_start(wenc_sb[:], wenc[:].rearrange("(c p) h -> p c h", p=128))
            wdec_sb = wp.tile([128, 5, H], BF16)
            nc.sync.dma_start(wdec_sb[:], wdeca[:].rearrange("(c p) h -> p c h", p=128))
            dech_sb = wp.tile([128, 5, BL], BF16)
            nc.sync.dma_start(dech_sb[:], dechTa[:].rearrange("(c p) b -> p c b", p=128))
            v_sb = wp.tile([128, 4], BF16)
            nc.sync.dma_start(v_sb[:], vcol[:].rearrange("(c p) -> p c", p=128))

            augL = wp.tile([2, BL * H], BF16)   # row0 dec proj (filled below), row1 w_cov
            augR = wp.tile([2, BL * S], BF16)   # row0 ones, row1 coverage
            nc.sync.dma_start(augL[1:2, :], wcov8[None, :])
            nc.sync.dma_start(augR[0:1, :], ones8k[None, :])
            nc.sync.dma_start(augR[1:2, :], covb[:].rearrange("b s -> (b s)")[None, :])

            # ---- dec projection (+bias): decp[b,h] = dec_h[b] @ W_dec + bias ----
            dps = pp.tile([BL, H], F32, tag="acc", bufs=2)
            for k in range(5):
                nc.tensor.matmul(
                    dps[:], dech_sb[:, k, :], wdec_sb[:, k, :],
                    start=(k == 0), stop=(k == 4),
                )
            decb = wp.tile([BL, H], BF16)
            nc.any.tensor_copy(decb[:], dps[:])
            for b in range(BL):
                nc.sync.dma_start(augL[0:1, b * H:(b + 1) * H], decb[b:b + 1, :])

            # ---- main per-batch pipeline ----
            # Emission order is software-pipelined so the PE stream never
            # waits on ACT/DVE chains: e-score matmuls trail their tanh
            # producers by a whole projection block, and batch b-1's h_star
            # matmuls are slotted into the middle of batch b's projections
            # (its softmax runs on DVE/ACT under the first block).

            def emit_proj_block(b, sh, et):
                ths = []
                for h in range(4):
                    prj = pp.tile([128, 512], F32, tag="prj", bufs=4)
                    for e in range(8):
                        nc.tensor.matmul(
                            prj[:],
                            wenc_sb[:, e, h * 128:(h + 1) * 128],
                            et[:, e, sh * 512:(sh + 1) * 512],
                            start=(e == 0), stop=False,
                        )
                    nc.tensor.matmul(
                        prj[:],
                        augL[:, b * H + h * 128: b * H + (h + 1) * 128],
                        augR[:, b * S + sh * 512: b * S + (sh + 1) * 512],
                        start=False, stop=True,
                    )
                    th = workp.tile([128, 512], BF16, tag="tanh", bufs=10)
                    nc.scalar.activation(th[:], prj[:], AF.Tanh)
                    ths.append(th)
                return ths

            def emit_escore(b, sh, ths, e_row):
                eps = pp.tile([1, 512], F32, tag="eps", bufs=2)
                for h in range(4):
                    nc.tensor.matmul(
                        eps[:], v_sb[:, h:h + 1], ths[h][:],
                        start=(h == 0), stop=(h == 3),
                    )
                nc.vector.tensor_copy(e_row[:, sh * 512:(sh + 1) * 512], eps[:])

            def emit_softmax(b, e_row):
                emul_r = rowp.tile([1, S], F32, tag="emul_r")
                nc.sync.dma_start(emul_r[:], emul[b:b + 1, :])
                eadd_r = rowp.tile([1, S], F32, tag="eadd_r")
                nc.sync.dma_start(eadd_r[:], eadd[b:b + 1, :])
                covf_r = rowp.tile([1, S], F32, tag="covf_r")
                nc.sync.dma_start(covf_r[:], covf[b:b + 1, :])
                em = rowp.tile([1, S], F32, tag="em")
                nc.vector.tensor_tensor(em[:], e_row[:], emul_r[:], op=ALU.mult)
                nc.vector.tensor_tensor(em[:], em[:], eadd_r[:], op=ALU.add)
                mx = rowp.tile([1, 1], F32, tag="mx")
                nc.vector.reduce_max(mx[:], em[:], axis=AX.X)
                nc.vector.tensor_scalar_mul(mx[:], mx[:], -1.0)
                pr = rowp.tile([1, S], F32, tag="pr")
                nc.scalar.activation(pr[:], em[:], AF.Exp, bias=mx[:, 0:1], scale=1.0)
                sm = rowp.tile([1, 1], F32, tag="sm")
                nc.vector.reduce_sum(sm[:], pr[:], axis=AX.X)
                rv = rowp.tile([1, 1], F32, tag="rv")
                nc.vector.reciprocal(rv[:], sm[:])
                ar = rowp.tile([1, S], F32, tag="ar")
                nc.vector.tensor_scalar_mul(ar[:], pr[:], rv[:, 0:1])
                nc.sync.dma_start(o_a[b:b + 1, :], ar[:])

                ncv = rowp.tile([1, S], F32, tag="ncv")
                nc.vector.tensor_tensor(ncv[:], ar[:], covf_r[:], op=ALU.add)
                nc.sync.dma_start(o_c[b:b + 1, :], ncv[:])
                mnr = rowp.tile([1, S], F32, tag="mnr")
                nc.vector.tensor_tensor(mnr[:], ar[:], covf_r[:], op=ALU.min)
                cl = rowp.tile([1, 1], F32, tag="cl")
                nc.vector.reduce_sum(cl[:], mnr[:], axis=AX.X)
                nc.sync.dma_start(o_l[b:b + 1, :], cl[:])

                # attention weights to stationary-column layout via DRAM bounce
                ab = rowp.tile([1, S], BF16, tag="ab")
                nc.vector.tensor_copy(ab[:], ar[:])
                scr = dp.tile([S], BF16, tag="scr")
                nc.sync.dma_start(scr[None, :], ab[:])
                aT = workp.tile([128, BL], BF16, tag="aT")
                nc.sync.dma_start(aT[:], scr[:].rearrange("(c p) -> p c", p=128))
                return aT

            def emit_hstar(b, aT):
                hp0 = pp.tile([1, 512], F32, tag="acc", bufs=2)
                hp1 = pp.tile([1, 512], F32, tag="acc", bufs=2)
                for sc in range(8):
                    en = workp.tile([128, E], BF16, tag="encN", bufs=6)
                    nc.sync.dma_start(en[:], encN[b, sc * 128:(sc + 1) * 128, :])
                    nc.tensor.matmul(hp0[:], aT[:, sc:sc + 1], en[:, 0:512],
                                     start=(sc == 0), stop=(sc == 7))
                    nc.tensor.matmul(hp1[:], aT[:, sc:sc + 1], en[:, 512:1024],
                                     start=(sc == 0), stop=(sc == 7))
                hs = rowp.tile([1, E], F32, tag="hs")
                nc.any.tensor_copy(hs[:, 0:512], hp0[:])
                nc.any.tensor_copy(hs[:, 512:1024], hp1[:])
                nc.sync.dma_start(o_h[b:b + 1, :], hs[:])

            pending = None  # (b-1, aT) awaiting h_star emission
            for b in range(BL):
                et = bigp.tile([128, 8, S], BF16, tag="encT")
                for e in range(8):
                    nc.sync.dma_start(et[:, e, :], encT[b, e * 128:(e + 1) * 128, :])

                e_row = rowp.tile([1, S], F32, tag="erow")
                ths0 = emit_proj_block(b, 0, et)
                if pending is not None:
                    emit_hstar(*pending)
                    pending = None
                ths1 = emit_proj_block(b, 1, et)
                emit_escore(b, 0, ths0, e_row)
                emit_escore(b, 1, ths1, e_row)
                aT = emit_softmax(b, e_row)
                pending = (b, aT)
            emit_hstar(*pending)

    return nc


def get_graph():
    global _graph
    if _graph is None:
        _graph = _build_graph()
        if not _graph.is_finalized():
            _graph.finalize()
    return _graph


def kernel(enc_h, dec_h, coverage_vec, mask, W_enc, W_dec, w_cov, bias, v):
    bf = ml_dtypes.bfloat16
    enc_h = np.asarray(enc_h)
    dec_h = np.asarray(dec_h, dtype=np.float32)
    coverage_vec = np.asarray(coverage_vec, dtype=np.float32)
    mask_f = np.asarray(mask).astype(np.float32)
    W_enc = np.asarray(W_enc, dtype=np.float32)
    W_dec = np.asarray(W_dec, dtype=np.float32)
    w_cov = np.asarray(w_cov, dtype=np.float32)
    bias = np.asarray(bias, dtype=np.float32)
    v = np.asarray(v, dtype=np.float32)

    enc_bf = enc_h.astype(bf)  # [B, S, E]

    wdeca = np.zeros((KD, H), np.float32)
    wdeca[:H] = W_dec
    wdeca[H] = bias[0]
    wdeca = wdeca.astype(bf)
    wenc_bf = W_enc.astype(bf)
    vcol = v.astype(bf)
    wcov8 = np.tile(w_cov, BL).astype(bf)
    ones8k = np.ones(BL * S, bf)

    in_maps = []
    for c in range(NCORES):
        sl = slice(c * BL, (c + 1) * BL)
        encN_c = np.ascontiguousarray(enc_bf[sl])
        encT_c = np.ascontiguousarray(encN_c.transpose(0, 2, 1))
        dechTa = np.zeros((KD, BL), np.float32)
        dechTa[:H] = dec_h[sl].T
        dechTa[H] = 1.0
        cov_c = np.ascontiguousarray(coverage_vec[sl])
        m_c = np.ascontiguousarray(mask_f[sl])
        in_maps.append({
            "encT": encT_c,
            "encN": encN_c,
            "wenc": wenc_bf,
            "wdeca": wdeca,
            "dechTa": dechTa.astype(bf),
            "vcol": vcol,
            "wcov8": wcov8,
            "ones8k": ones8k,
            "covb": cov_c.astype(bf),
            "covf": cov_c,
            "emul": m_c,
            "eadd": ((m_c - 1.0) * 1e9).astype(np.float32),
        })

    run = run_bass_kernel_spmd(get_graph(), in_maps, list(range(NCORES)))
    global LAST_RUN
    LAST_RUN = run
    res = run.results

    a = np.concatenate([np.asarray(r["o_a"], np.float32) for r in res], axis=0)
    h_star = np.concatenate([np.asarray(r["o_h"], np.float32) for r in res], axis=0)
    new_cov = np.concatenate([np.asarray(r["o_c"], np.float32) for r in res], axis=0)
    covloss = np.concatenate(
        [np.asarray(r["o_l"], np.float32).reshape(BL) for r in res], axis=0
    )
    return (a, h_star, new_cov, covloss)


# revision 14
# speedup vs baseline: 1.3931x; 1.2212x over previous
"""Trainium2 Bass kernel for coverage-attention (Bahdanau-style with coverage).

Reference computation (per batch row b):
  proj_sum[s,h] = enc_h[b,s,:] @ W_enc[:,h] + (dec_h[b] @ W_dec)[h] + cov[b,s]*w_cov[h]
  e[s]     = sum_h tanh(proj_sum[s,h] + bias) * v[h]
  a        = softmax(where(mask, e, -1e9))
  covloss  = sum_s min(a, cov)
  h_star   = sum_s a[s] * enc_h[b,s,:]
  new_cov  = cov + a

Strategy: data-parallel over batch (64 -> 8 per NeuronCore). Compute in bf16
on the TensorEngine (PSUM accumulates f32). The host pre-shards and
pre-marshals layouts: enc_h is shipped both natural [b,s,e] (for h_star) and
transposed [b,e,s] (for the projection contraction over e), already cast to
bf16. The dec-projection + bias and the coverage outer-product are folded
into the matmul chain as augmented rank-2 terms. proj is computed transposed
([h_part, s_free]) so the v-weighted tanh reduction is a TensorE matmul.
"""

import sys

sys.path.insert(0, "/opt/trn_rl_repo")

import numpy as np
import ml_dtypes

import concourse.bass as bass
import concourse.mybir as mybir
import concourse.tile as tile
from concourse import bacc
from concourse.bass_utils import run_bass_kernel_spmd

B, S, E, H = 64, 1024, 1024, 512  # E = 2H
NCORES = 8
BL = B // NCORES  # 8 local batch rows per core
KD = 640  # padded contraction dim for dec proj (512 + 1 bias row, padded to 5*128)

BF16 = mybir.dt.bfloat16
F32 = mybir.dt.float32
AF = mybir.ActivationFunctionType
ALU = mybir.AluOpType
AX = mybir.AxisListType

_graphs = {}
LAST_RUN = None  # BassKernelResults of the most recent run (exec_time_ns when traced)


def _build_graph(masked):
    """masked=False is the fast path used when mask is all-ones (the spec's
    fill); masked=True applies the where(mask, e, -1e9) select generally."""
    nc = bacc.Bacc()
    P = nc.declare_dram_parameter
    encT = P("encT", [BL, E, S], BF16, False)   # enc' = enc + cov x u, transposed
    encN = P("encN", [BL, S, E], BF16, False)   # original enc, natural layout
    wenc = P("wenc", [E, H], BF16, False)
    wdeca = P("wdeca", [KD, H], BF16, False)       # rows 0..511 W_dec, 512 bias, rest 0
    dechTa = P("dechTa", [KD, BL], BF16, False)    # rows 0..511 dec_h.T, 512 ones, rest 0
    vcol = P("vcol", [H], BF16, False)
    covf = P("covf", [BL, S], F32, False)
    if masked:
        emul = P("emul", [BL, S], F32, False)
        eadd = P("eadd", [BL, S], F32, False)
    o_a = P("o_a", [BL, S], F32, True)
    o_h = P("o_h", [BL, E], F32, True)
    o_c = P("o_c", [BL, S], F32, True)
    o_l = P("o_l", [BL, 1], F32, True)

    with tile.TileContext(nc) as tc:
        with (
            tc.tile_pool(name="wp", bufs=1) as wp,
            tc.tile_pool(name="bigp", bufs=1) as bigp,
            tc.tile_pool(name="workp", bufs=3) as workp,
            tc.tile_pool(name="rowp", bufs=2) as rowp,
            tc.tile_pool(name="psum", bufs=1, space="PSUM") as pp,
            tc.tile_pool(name="dramp", bufs=2, space="DRAM") as dp,
        ):
            # ---- persistent weights ----
            wenc_sb = wp.tile([128, 8, H], BF16)
            nc.sync.dma_start(wenc_sb[:], wenc[:].rearrange("(c p) h -> p c h", p=128))
            wdec_sb = wp.tile([128, 5, H], BF16)
            nc.sync.dma_start(wdec_sb[:], wdeca[:].rearrange("(c p) h -> p c h", p=128))
            dech_sb = wp.tile([128, 5, BL], BF16)
            nc.sync.dma_start(dech_sb[:], dechTa[:].rearrange("(c p) b -> p c b", p=128))
            v_sb = wp.tile([128, 4], BF16)
            nc.sync.dma_start(v_sb[:], vcol[:].rearrange("(c p) -> p c", p=128))

            # ---- transposed dec projection: dec_colT[p, c, b] = (dec_h[b] @ W_dec + bias)[c*128+p]
            dec_colT = wp.tile([128, 4, BL], F32)
            for c in range(4):
                dtp = pp.tile([128, BL], F32, tag="eps", bufs=2)
                for k in range(5):
                    nc.tensor.matmul(
                        dtp[:], wdec_sb[:, k, c * 128:(c + 1) * 128], dech_sb[:, k, :],
                        start=(k == 0), stop=(k == 4),
                    )
                nc.vector.tensor_copy(dec_colT[:, c, :], dtp[:])

            # ---- emission helpers (software-pipelined schedule) ----
            def emit_proj_block(b, sh, ets):
                ths = []
                for h in range(4):
                    prj = pp.tile([128, 512], F32, tag="prj", bufs=4)
                    for e in range(8):
                        nc.tensor.matmul(
                            prj[:],
                            wenc_sb[:, e, h * 128:(h + 1) * 128],
                            ets[e][:, sh * 512:(sh + 1) * 512],
                            start=(e == 0), stop=(e == 7),
                        )
                    th = workp.tile([128, 512], BF16, tag="tanh", bufs=10)
                    nc.scalar.activation(th[:], prj[:], AF.Tanh,
                                         bias=dec_colT[:, h, b:b + 1])
                    ths.append(th)
                return ths

            def emit_escore(b, sh, ths, eps_out):
                for h in range(4):
                    nc.tensor.matmul(
                        eps_out[:], v_sb[:, h:h + 1], ths[h][:],
                        start=(h == 0), stop=(h == 3),
                    )

            def emit_softmax(b, eps0, eps1):
                """exp + transpose-to-columns; returns (aT_unnormalized, rv, p_row)."""
                p_row = rowp.tile([1, S], BF16, tag="p_row")
                if masked:
                    emul_r = rowp.tile([1, S], F32, tag="emul_r")
                    nc.sync.dma_start(emul_r[:], emul[b:b + 1, :])
                    eadd_r = rowp.tile([1, S], F32, tag="eadd_r")
                    nc.sync.dma_start(eadd_r[:], eadd[b:b + 1, :])
                    em = rowp.tile([1, S], F32, tag="em")
                    nc.vector.tensor_copy(em[:, 0:512], eps0[:])
                    nc.vector.tensor_copy(em[:, 512:1024], eps1[:])
                    nc.vector.tensor_tensor(em[:], em[:], emul_r[:], op=ALU.mult)
                    nc.vector.tensor_tensor(em[:], em[:], eadd_r[:], op=ALU.add)
                    mx = rowp.tile([1, 1], F32, tag="mx")
                    nc.vector.reduce_max(mx[:], em[:], axis=AX.X)
                    nc.vector.tensor_scalar_mul(mx[:], mx[:], -1.0)
                    nc.scalar.activation(p_row[:], em[:], AF.Exp, bias=mx[:, 0:1])
                else:
                    # scores are O(1) here so exp needs no max-subtraction;
                    # read the e-score PSUM tiles directly.
                    nc.scalar.activation(p_row[:, 0:512], eps0[:], AF.Exp)
                    nc.scalar.activation(p_row[:, 512:1024], eps1[:], AF.Exp)
                sm = rowp.tile([1, 1], F32, tag="sm")
                nc.vector.reduce_sum(sm[:], p_row[:], axis=AX.X)
                rv = rowp.tile([1, 1], F32, tag="rv")
                nc.vector.reciprocal(rv[:], sm[:])
                # unnormalized attention columns via DRAM bounce
                scr = dp.tile([S], BF16, tag="scr")
                nc.sync.dma_start(scr[None, :], p_row[:])
                aT = workp.tile([128, BL], BF16, tag="aT")
                nc.sync.dma_start(aT[:], scr[:].rearrange("(c p) -> p c", p=128))
                return aT, rv, p_row

            def emit_outputs(b, p_row, rv):
                """a, new_coverage, covloss — off the PE critical path."""
                covf_r = rowp.tile([1, S], F32, tag="covf_r")
                nc.sync.dma_start(covf_r[:], covf[b:b + 1, :])
                ar = rowp.tile([1, S], F32, tag="ar")
                nc.vector.tensor_scalar_mul(ar[:], p_row[:], rv[:, 0:1])
                nc.sync.dma_start(o_a[b:b + 1, :], ar[:])
                ncv = rowp.tile([1, S], F32, tag="ncv")
                nc.vector.tensor_tensor(ncv[:], ar[:], covf_r[:], op=ALU.add)
                nc.sync.dma_start(o_c[b:b + 1, :], ncv[:])
                mnr = rowp.tile([1, S], F32, tag="mnr")
                nc.vector.tensor_tensor(mnr[:], ar[:], covf_r[:], op=ALU.min)
                cl = rowp.tile([1, 1], F32, tag="cl")
                nc.vector.reduce_sum(cl[:], mnr[:], axis=AX.X)
                nc.sync.dma_start(o_l[b:b + 1, :], cl[:])

            def emit_hstar(b, aT, rv):
                hp0 = pp.tile([1, 512], F32, tag="acc", bufs=2)
                hp1 = pp.tile([1, 512], F32, tag="acc", bufs=2)
                for sc in range(8):
                    en = workp.tile([128, E], BF16, tag="encN", bufs=6)
                    nc.sync.dma_start(en[:], encN[b, sc * 128:(sc + 1) * 128, :])
                    nc.tensor.matmul(hp0[:], aT[:, sc:sc + 1], en[:, 0:512],
                                     start=(sc == 0), stop=(sc == 7))
                    nc.tensor.matmul(hp1[:], aT[:, sc:sc + 1], en[:, 512:1024],
                                     start=(sc == 0), stop=(sc == 7))
                hs = rowp.tile([1, E], F32, tag="hs")
                nc.vector.tensor_scalar_mul(hs[:, 0:512], hp0[:], rv[:, 0:1])
                nc.vector.tensor_scalar_mul(hs[:, 512:1024], hp1[:], rv[:, 0:1])
                nc.sync.dma_start(o_h[b:b + 1, :], hs[:])

            def load_encT(b):
                ets = []
                for e in range(8):
                    et = bigp.tile([128, S], BF16, tag="encT", bufs=18)
                    nc.sync.dma_start(et[:], encT[b, e * 128:(e + 1) * 128, :])
                    ets.append(et)
                return ets

            # ---- main per-batch pipeline ----
            pending = None  # (b-1, aT, rv) awaiting h_star emission
            for b in range(BL):
                ets = load_encT(b)
                eps0 = pp.tile([1, 512], F32, tag="eps", bufs=2)
                eps1 = pp.tile([1, 512], F32, tag="eps", bufs=2)
                ths0 = emit_proj_block(b, 0, ets)
                if pending is not None:
                    emit_hstar(*pending)
                    pending = None
                ths1 = emit_proj_block(b, 1, ets)
                emit_escore(b, 0, ths0, eps0)
                emit_escore(b, 1, ths1, eps1)
                aT, rv, p_row = emit_softmax(b, eps0, eps1)
                emit_outputs(b, p_row, rv)
                pending = (b, aT, rv)
            emit_hstar(*pending)

    return nc


def get_graph(masked):
    global _graphs
    if _graphs.get(masked) is None:
        g = _build_graph(masked)
        if not g.is_finalized():
            g.finalize()
        _graphs[masked] = g
    return _graphs[masked]


def kernel(enc_h, dec_h, coverage_vec, mask, W_enc, W_dec, w_cov, bias, v):
    bf = ml_dtypes.bfloat16
    enc_h = np.asarray(enc_h)
    dec_h = np.asarray(dec_h, dtype=np.float32)
    coverage_vec = np.asarray(coverage_vec, dtype=np.float32)
    mask_b = np.asarray(mask).astype(bool)
    W_enc = np.asarray(W_enc, dtype=np.float32)
    W_dec = np.asarray(W_dec, dtype=np.float32)
    w_cov = np.asarray(w_cov, dtype=np.float32)
    bias = np.asarray(bias, dtype=np.float32)
    v = np.asarray(v, dtype=np.float32)

    # Fast path: all-ones mask (the spec's fill) and scores that cannot
    # overflow exp without max-subtraction.
    masked = (not bool(mask_b.all())) or float(np.abs(v).sum()) > 80.0

    # Fold the coverage outer-product cov[s]*w_cov[h] into the main
    # contraction: find least-norm u with W_enc.T @ u = w_cov, then
    # enc' = enc + cov x u satisfies enc' @ W_enc = enc @ W_enc + cov x w_cov.
    W64 = W_enc.astype(np.float64)
    u = (W64 @ np.linalg.solve(W64.T @ W64, w_cov.astype(np.float64))).astype(np.float32)

    wdeca = np.zeros((KD, H), np.float32)
    wdeca[:H] = W_dec
    wdeca[H] = bias[0]
    wdeca = wdeca.astype(bf)
    wenc_bf = W_enc.astype(bf)
    vcol = v.astype(bf)

    mask_f = mask_b.astype(np.float32)
    in_maps = []
    for c in range(NCORES):
        sl = slice(c * BL, (c + 1) * BL)
        enc_c = np.asarray(enc_h[sl], dtype=np.float32)
        cov_c = np.ascontiguousarray(coverage_vec[sl])
        encN_c = enc_c.astype(bf)
        encT_c = np.ascontiguousarray(
            (enc_c + cov_c[:, :, None] * u[None, None, :]).transpose(0, 2, 1)
        ).astype(bf)
        dechTa = np.zeros((KD, BL), np.float32)
        dechTa[:H] = dec_h[sl].T
        dechTa[H] = 1.0
        m = {
            "encT": encT_c,
            "encN": encN_c,
            "wenc": wenc_bf,
            "wdeca": wdeca,
            "dechTa": dechTa.astype(bf),
            "vcol": vcol,
            "covf": cov_c,
        }
        if masked:
            m_c = np.ascontiguousarray(mask_f[sl])
            m["emul"] = m_c
            m["eadd"] = ((m_c - 1.0) * 1e9).astype(np.float32)
        in_maps.append(m)

    run = run_bass_kernel_spmd(get_graph(masked), in_maps, list(range(NCORES)))
    global LAST_RUN
    LAST_RUN = run
    res = run.results

    a = np.concatenate([np.asarray(r["o_a"], np.float32) for r in res], axis=0)
    h_star = np.concatenate([np.asarray(r["o_h"], np.float32) for r in res], axis=0)
    new_cov = np.concatenate([np.asarray(r["o_c"], np.float32) for r in res], axis=0)
    covloss = np.concatenate(
        [np.asarray(r["o_l"], np.float32).reshape(BL) for r in res], axis=0
    )
    return (a, h_star, new_cov, covloss)
